# revision 12
# baseline (speedup 1.0000x reference)
"""Trainium2 Bass kernel for nn_Attention_89670327206161 (Gram restructure).

The reference contracts attention scores over the *sequence* axis, so per
head the score matrix is only (dh x dh) = 64x64:
    scores_h = K_h^T Q_h / 8 = Wk_h (x^T x) Wq_h^T / 8
    out      = x . Wv^T . blockdiag(softmax(scores)) . Wo^T
The whole layer therefore collapses to GEMMs around one 1024x1024 Gram
matrix instead of three projections + attention + out-projection:
    G  = x^T x          (symmetric: pass A = cols 0:512 all rows,
                         pass B = bottom-right quadrant, top-right
                         quadrant mirrored with PE transposes)
    T2 = G Wq^T         (G's symmetry supplies the lhsT blocks; the 1/8
                         score scale is folded into the wq upload)
    scores_p = Wk_pair T2   (256-wide rhs keeps fp32r at full rate)
    P  = softmax_rows(scores)   (max-subtracted exp per 64x64 block;
                                 exp's accum_out gives the row sum free)
    U^T = BD(P)^T Wv-rows;  M = U Wo^T;  outT = M-blocks^T @ xT
The pre-softmax path (G, T2, scores) runs in fp32r -- softmax amplifies
logit error (|logits| reach ~140) so bf16 there is fatal. The
post-softmax path (P, Wv, U, Wo, M, xT) is plain linear algebra with
plenty of tolerance headroom, so it runs in bf16: 1024-wide moving
operands, half the DMA bytes, LDWEIGHTS hidden under longer matmuls.

Sharding: pure data parallelism -- one batch element per core, no
collectives. Host supplies x twice (seq-major fp32 for G, feature-major
bf16 for the final pass); output returns feature-major fp32, transposed
on host.

DMA choreography (the xa stream must never starve; TileContext schedules
by data deps only, so WAW gate copies into each DMA's destination pin
transfer start times):
  sync queue   : xa seq tiles, xb half tiles, xt blocks, out blocks.
  scalar queue : wq half0 behind xa tile 26; wq half1 + wk + wv + wo +
                 xt0 behind pass B's first copy-out -- all land inside
                 T2's DMA-quiet window, before their consumers.
  gpsimd queue : only gate copies and softmax scale muls, so the softmax
                 critical path is never queued behind DMA work.
"""

import numpy as np

HEADS = 16
B, S, D = 8, 4096, 1024
P = 128
NKC = D // P             # 8 chunks of 128 along D
NT = S // P              # 32 seq tiles
NPAIR = HEADS // 2       # 8 head pairs -> 128-wide blocks
N_CORES = 8

_PROGRAM = None


def _ts(i, n):
    return slice(i * n, (i + 1) * n)


def _build_program(reps=1):
    import concourse.bacc as bacc
    import concourse.mybir as mybir
    import concourse.tile as tile
    from concourse.masks import make_identity

    f32 = mybir.dt.float32
    f32r = mybir.dt.float32r
    bf16 = mybir.dt.bfloat16
    EXP = mybir.ActivationFunctionType.Exp
    X = mybir.AxisListType.X

    nc = bacc.Bacc(trn_type="TRN2", debug=False, num_devices=N_CORES)

    # weights and xT arrive pre-packed in SBUF layout ([partition, ...]
    # with long contiguous per-partition rows): the natural "(c p) o"
    # rearrangement produces 512-byte DMA descriptors, which run
    # descriptor-rate-bound at ~21 GB/s (measured: 2 MB of wq took
    # 11.5 us across 4160 descriptors)
    xs = nc.dram_tensor("xs", [S, D], f32r, kind="ExternalInput")
    xT = nc.dram_tensor("xT", [NKC, P, NKC * 512], bf16, kind="ExternalInput")
    wqT = nc.dram_tensor("wqT", [P, NKC * D], f32r, kind="ExternalInput")
    wkT = nc.dram_tensor("wkT", [P, NKC * D], f32r, kind="ExternalInput")
    wv = nc.dram_tensor("wv", [P, NKC * D], bf16, kind="ExternalInput")
    woT = nc.dram_tensor("woT", [P, NKC * D], bf16, kind="ExternalInput")
    outT = nc.dram_tensor("outT", [D, S], f32, kind="ExternalOutput")

    xs_ap = xs.ap()
    xTr = xT.ap().rearrange("b p (c s) -> b p c s", c=NKC)
    wqTr = wqT.ap().rearrange("p (c o) -> p c o", c=NKC)
    wkTr = wkT.ap().rearrange("p (c o) -> p c o", c=NKC)
    wvr = wv.ap().rearrange("p (r c) -> p r c", r=NKC)
    woTr = woT.ap().rearrange("p (c o) -> p c o", c=NKC)
    outTr = outT.ap().rearrange("(c p) s -> p c s", p=P)

    with tile.TileContext(nc) as tc:
      with (
          tc.tile_pool(name="const", bufs=1) as const_pool,
          tc.tile_pool(name="persist", bufs=1) as persist_pool,
          tc.tile_pool(name="smx", bufs=4) as smx_pool,
          tc.tile_pool(name="t2u", bufs=1) as t2u_pool,
          tc.tile_pool(name="arena", bufs=1) as arena,
          tc.tile_pool(name="wvwo", bufs=1) as wvwo_pool,
      ):
        zero_sb = const_pool.tile([P, 512], f32r, tag="zero")
        ident_raw = const_pool.tile([P, P], f32, tag="identr")
        ident = const_pool.tile([P, P], f32r, tag="ident")

        nc.vector.memset(zero_sb[:].bitcast(f32), 0.0)
        # affine_select output isn't fp32r-rounded for the BIR verifier;
        # route it through a copy, which is
        make_identity(nc, ident_raw[:])
        nc.vector.tensor_copy(ident[:], ident_raw[:])

        # reps>1 re-executes the whole body (timing builds: the difference
        # between reps=2 and reps=1 cancels dispatch overhead exactly)
        for _rep in range(reps):
          p_all = persist_pool.tile([P, NPAIR, P], bf16, tag="pall")
          nc.vector.memset(p_all[:], 0.0)
          g_sb = arena.tile([P, NKC, D], f32r, tag="a")
          t2_sb = t2u_pool.tile([P, NKC, D], f32r, tag="t2")
          wv_sb = wvwo_pool.tile([P, NKC, D], bf16, tag="wv")
          wo_sb = wvwo_pool.tile([P, NKC, D], bf16, tag="wo")
          ut_sb = wvwo_pool.tile([P, NKC, D], bf16, tag="ut")

          with (
              tc.tile_pool(name="wq", bufs=1) as wq_pool,
              tc.tile_pool(name="wk", bufs=1) as wk_pool,
          ):
            wq_sb = wq_pool.tile([P, NKC, D], f32r, tag="wq")
            wk_sb = wk_pool.tile([P, NKC, D], f32r, tag="wk")

            # Triangular G: only lower blocks (i >= j) are computed; the
            # upper triangle is mirrored with PE transposes.  fp32r MMs
            # below 256-wide run at 1/4 rate, so the two narrowest rows
            # are widened to 256 (their extra 128-col block lands on the
            # (0,1)/(4,5) upper blocks directly, skipping those mirrors).
            WA = [256, 256, 384, 512, 512, 512, 512, 512]
            OA = [0, 256, 512, 1024, 1536, 2048, 2560, 3072]  # 7 banks
            WB = [256, 256, 384, 512]
            OB = [0, 256, 512, 1024]  # 3 banks
            # top-left-quadrant mirrors go first: T2 h0 groups m=0..3
            # depend only on these (+ pass A), and are emitted right
            # after pass B's matmuls to cover the copy-out latency
            EARLY = [(c, m) for m in range(4, 8) for c in range(4)] + [
                (0, 2), (1, 2), (0, 3), (1, 3), (2, 3)]
            LATE = [(4, 6), (4, 7), (5, 6), (5, 7), (6, 7)]  # pass B srcs
            with (
                tc.tile_pool(name="xa", bufs=4) as xa_pool,
                tc.tile_pool(name="xbp", bufs=4) as xb_pool,
            ):
                with tc.tile_pool(name="gA", bufs=1, space="PSUM") as gA_pool:
                    # ---- G pass A: lower-triangle cols 0:512, all rows
                    g_ps = gA_pool.tile([P, 7 * 512], f32, tag="gps")
                    # HAM warm-up + has_written clear: one dummy per bank
                    for b in range(7):
                        nc.tensor.matmul(
                            g_ps[:, _ts(b, 512)], zero_sb[:, 0:P], zero_sb[:],
                            start=True, stop=False, skip_group_check=True,
                        )
                    for st in range(NT):
                        xa = xa_pool.tile([P, D], f32r, tag="xa")
                        nc.sync.dma_start(xa[:], xs_ap[_ts(st, P), :])
                        for ci in range(NKC):
                            nc.tensor.matmul(
                                g_ps[:, OA[ci]:OA[ci] + WA[ci]],
                                xa[:, _ts(ci, P)], xa[:, 0:WA[ci]],
                                start=False, stop=(st == NT - 1),
                                skip_group_check=True,
                            )
                        if st == 26:
                            # WAW gate (hoist protection): wq half0 waits
                            # for tile 26, then rides the sync queue right
                            # behind the xa stream
                            nc.gpsimd.tensor_copy(
                                wq_sb[0:1, 0, 0:512], xa[0:1, 0:512])
                    # scalar queue: keeps the sync queue free for the
                    # xb stream (pass A's xa stream already saturates
                    # it; 2MB of wq here would stall pass B's start)
                    nc.scalar.dma_start(wq_sb[:, :, 0:512], wqTr[:, :, 0:512])
                    # drain order frees the PSUM banks that pass B's
                    # pools alias first, so its matmuls start ~3us sooner
                    # (the copies serialize S<->V at ~690ns each)
                    for k, ci in enumerate([4, 5, 6, 7, 0, 1, 2, 3]):
                        eng = nc.scalar.copy if k % 2 == 0 else nc.vector.tensor_copy
                        eng(g_sb[:, ci, 0:WA[ci]], g_ps[:, OA[ci]:OA[ci] + WA[ci]])

                # ---- G pass B: lower-triangle cols 512:1024, rows 4-7,
                # with the pass-A-sourced mirrors riding the PE between
                # the DMA-paced pass-B tiles.  The T2 psum pool opens
                # here (3+3+2 = 8 banks) so the first T2 h0 groups can
                # fill the PE while pass B's copy-outs/gates drain.
                with tc.tile_pool(name="big", bufs=3, space="PSUM") as big_pool:

                  def t2_group(h, m):
                      ps = big_pool.tile([P, 512], f32, tag="bps")
                      for c in range(NKC):
                          nc.tensor.matmul(
                              ps[:], g_sb[:, c, _ts(m, P)],
                              wq_sb[:, c, _ts(h, 512)],
                              start=(c == 0), stop=(c == NKC - 1),
                          )
                      eng = nc.scalar.copy if m % 2 == 0 else nc.vector.tensor_copy
                      eng(t2_sb[:, m, _ts(h, 512)], ps[:])

                  with (
                    tc.tile_pool(name="gB", bufs=1, space="PSUM") as gB_pool,
                    tc.tile_pool(name="tr", bufs=2, space="PSUM") as tr_pool,
                  ):
                    def mirror(c, m, k):
                        # g_sb[:, c, m-block] = (g_sb[:, m, c-block])^T
                        t_ps = tr_pool.tile([P, P], f32r, tag="tr")
                        nc.tensor.transpose(
                            t_ps[:], g_sb[:, m, _ts(c, P)], ident[:])
                        eng = nc.scalar.copy if k % 2 == 0 else nc.vector.tensor_copy
                        eng(g_sb[:, c, _ts(m, P)], t_ps[:])

                    gb_ps = gB_pool.tile([P, 1536], f32, tag="gbps")
                    # start=True clears has_written for the WHOLE bank, so
                    # banks shared by two row-chunks must be dummy-cleared
                    # once and then only accumulated into (start=False)
                    for b in range(3):
                        nc.tensor.matmul(
                            gb_ps[:, _ts(b, 512)], zero_sb[:, 0:P], zero_sb[:],
                            start=True, stop=False, skip_group_check=True,
                        )
                    mi = 0
                    for st in range(NT):
                        xb = xb_pool.tile([P, 512], f32r, tag="xb")
                        nc.sync.dma_start(xb[:], xs_ap[_ts(st, P), 512:D])
                        for j in range(4):
                            nc.tensor.matmul(
                                gb_ps[:, OB[j]:OB[j] + WB[j]],
                                xb[:, _ts(j, P)], xb[:, 0:WB[j]],
                                start=False, stop=(st == NT - 1),
                                skip_group_check=True,
                            )
                        if st >= 4 and mi < len(EARLY):
                            mirror(*EARLY[mi], mi)
                            mi += 1
                    # copy-outs, chunk 7 first: the weight-DMA gates key
                    # on it, and chunk 7 is never a mirror destination
                    for k, j in enumerate([3, 0, 1, 2]):
                        eng = nc.scalar.copy if k % 2 == 0 else nc.vector.tensor_copy
                        eng(g_sb[:, 4 + j, 512:512 + WB[j]],
                            gb_ps[:, OB[j]:OB[j] + WB[j]])

                    # WAW gates on chunk 7's copy-out: the remaining
                    # weights + xt0 flow during T2's DMA-quiet window
                    nc.gpsimd.tensor_copy(
                        wq_sb[0:1, 0, 512:D], g_sb[0:1, 7, 512:D])
                    nc.gpsimd.tensor_copy(
                        wk_sb[0:1, 0, :], g_sb[0:1, 7, :])
                    nc.gpsimd.tensor_copy(
                        wv_sb[0:1, 0, :], g_sb[0:1, 7, :])
                    nc.gpsimd.tensor_copy(
                        wo_sb[0:1, 0, :], g_sb[0:1, 7, :])
                    nc.sync.dma_start(wq_sb[:, :, 512:D], wqTr[:, :, 512:D])
                    nc.sync.dma_start(wk_sb[:, :, 0:512], wkTr[:, :, 0:512])
                    nc.sync.dma_start(wk_sb[:, :, 512:D], wkTr[:, :, 512:D])
                    nc.sync.dma_start(wv_sb[:], wvr[:])
                    nc.sync.dma_start(wo_sb[:], woTr[:])

                    # T2 h0 m=0..3 (pass A + quadrant mirrors only): PE
                    # work while the copy-outs, gates and weight DMAs run
                    for m in range(4):
                        t2_group(0, m)

                    for k, cm in enumerate(LATE):
                        mirror(*cm, k)

                  # ---- T2 / scores / softmax era
                  with tc.tile_pool(name="sc", bufs=4, space="PSUM") as sc_pool:
                    for m in range(4, NKC):
                        t2_group(0, m)

                    def softmax_half(ps, off, p, hf, rec):
                        # scores arrive pre-scaled by 1/8 but still reach
                        # |149|, so per-row max subtraction is mandatory.
                        # exp's accum_out yields the row sum for free.
                        # exp writes the UNNORMALIZED weights straight to
                        # p_all; the 1/rowsum lands on wv instead (one
                        # in-place scale per pair) so no per-block
                        # normalize op sits on the softmax critical path
                        # (the old gpsimd mul cost 1.16us per call and
                        # serialized the whole era).
                        rows = slice(64 * hf, 64 * hf + 64)
                        cols = slice(off + 64 * hf, off + 64 * hf + 64)
                        mx = smx_pool.tile([P, 1], f32, tag="mx")
                        nc.vector.reduce_max(mx[rows, 0:1], ps[rows, cols], axis=X, negate=True)
                        den = smx_pool.tile([P, 1], f32, tag="den")
                        nc.scalar.activation(
                            p_all[rows, p, 64 * hf:64 * hf + 64],
                            ps[rows, cols], EXP,
                            bias=mx[rows, 0:1], accum_out=den[rows, 0:1],
                        )
                        nc.vector.reciprocal(rec[rows, 0:1], den[rows, 0:1])

                    def ut_pair(p):
                        # U^T[pair-rows] = E_pair^T @ (diag(rec) Wv)[pair];
                        # emitted one pair behind softmax so the PE never
                        # waits on the softmax chain
                        for h in range(2):
                            ps = big_pool.tile([P, 512], f32, tag="bps")
                            nc.tensor.matmul(
                                ps[:], p_all[:, p, :], wv_sb[:, p, _ts(h, 512)],
                                start=True, stop=True,
                            )
                            eng = nc.scalar.copy if h == 0 else nc.vector.tensor_copy
                            eng(ut_sb[:, p, _ts(h, 512)], ps[:])

                    def scores_softmax(p):
                        c0 = min(p * P, D - 256)
                        ps = sc_pool.tile([P, 256], f32, tag="sc")
                        for m in range(NKC):
                            nc.tensor.matmul(
                                ps[:], wk_sb[:, m, _ts(p, P)],
                                t2_sb[:, m, c0:c0 + 256],
                                start=(m == 0), stop=(m == NKC - 1),
                            )
                        off = p * P - c0
                        rec = smx_pool.tile([P, 1], f32, tag="rec")
                        softmax_half(ps, off, p, 0, rec)
                        softmax_half(ps, off, p, 1, rec)
                        # fold the softmax row normalization into this
                        # pair's private wv rows (in place, off the PE)
                        nc.vector.tensor_scalar_mul(
                            wv_sb[:, p, :], wv_sb[:, p, :], rec[:, 0:1])

                    # T2 h1 with pairs 0-3 interleaved: their softmax chains
                    # finish under T2 compute instead of stalling U^T later
                    for m in range(NKC):
                        t2_group(1, m)
                        if m % 2 == 1:
                            scores_softmax(m // 2)
                    for p in range(4, NPAIR):
                        scores_softmax(p)
                        ut_pair(p - 4)
                    for p in range(4, NPAIR):
                        ut_pair(p)

          # ---- M / outT era (bf16 operands; PSUM outputs are
          # bank-limited to 512 fp32, so groups stay 512 wide)
          m_sb = arena.tile([P, NKC, D], bf16, tag="a")
          with (
              tc.tile_pool(name="xt", bufs=2) as xt_pool,
              tc.tile_pool(name="ot", bufs=8) as ot_pool,
              tc.tile_pool(name="bigB", bufs=4, space="PSUM") as bigB_pool,
          ):
              # xt0 gated (WAW) on T2h0's copy-out
              xt0 = xt_pool.tile([P, NKC, 512], bf16, tag="xt")
              nc.gpsimd.tensor_copy(xt0[0:1, 0, 0:1], t2_sb[0:1, 0, 0:1])
              nc.sync.dma_start(xt0[:], xTr[0])

              # ---- M = U @ Wo^T
              for h in range(2):
                  for a in range(NKC):
                      ps = bigB_pool.tile([P, 512], f32, tag="bps")
                      for c in range(NKC):
                          nc.tensor.matmul(
                              ps[:], ut_sb[:, c, _ts(a, P)],
                              wo_sb[:, c, _ts(h, 512)],
                              start=(c == 0), stop=(c == NKC - 1),
                          )
                      eng = nc.scalar.copy if a % 2 == 0 else nc.vector.tensor_copy
                      eng(m_sb[:, a, _ts(h, 512)], ps[:])

              # ---- outT = M-blocks^T @ xT, streamed in 512-seq blocks
              for sb in range(NKC):
                  if sb == 0:
                      xt = xt0
                  else:
                      xt = xt_pool.tile([P, NKC, 512], bf16, tag="xt")
                      nc.sync.dma_start(xt[:], xTr[sb])
                  for oc in range(NKC):
                      ps = bigB_pool.tile([P, 512], f32, tag="bps")
                      for ci in range(NKC):
                          nc.tensor.matmul(
                              ps[:], m_sb[:, ci, _ts(oc, P)], xt[:, ci, :],
                              start=(ci == 0), stop=(ci == NKC - 1),
                          )
                      ot = ot_pool.tile([P, 512], f32, tag="ot")
                      eng = nc.scalar.copy if oc % 2 == 0 else nc.vector.tensor_copy
                      eng(ot[:], ps[:])
                      # two queues: one lags ~2 transfers behind the MM
                      # stream and exposes the drain at the kernel tail
                      dq = nc.sync if oc % 2 == 0 else nc.scalar
                      dq.dma_start(outTr[:, oc, _ts(sb, 512)], ot[:])

    nc.compile()
    return nc


def _get_program():
    global _PROGRAM
    if _PROGRAM is None:
        _PROGRAM = _build_program()
    return _PROGRAM


def _pack_pcf(a):
    # [(c p), f] -> [p, (c f)]: SBUF layout, long contiguous DMA rows
    c = a.shape[0] // P
    return np.ascontiguousarray(
        a.reshape(c, P, a.shape[1]).transpose(1, 0, 2).reshape(P, -1))


def prep_inputs(x, Wq, Wk, Wv, Wo):
    import ml_dtypes

    x = np.asarray(x, np.float32)
    xs_all = np.ascontiguousarray(x)
    # xT packed per 512-seq block: [sb, p, (c s)]
    xT_all = np.ascontiguousarray(
        np.transpose(x, (0, 2, 1)).astype(ml_dtypes.bfloat16)
        .reshape(B, NKC, P, NKC, 512).transpose(0, 3, 2, 1, 4)
        .reshape(B, NKC, P, NKC * 512))
    # 1/8 score scale folded into wq so softmax needs no scale pass
    wqT = _pack_pcf(np.asarray(Wq, np.float32).T * np.float32(0.125))
    wkT = _pack_pcf(np.asarray(Wk, np.float32).T)
    wv_ = _pack_pcf(np.asarray(Wv, np.float32).astype(ml_dtypes.bfloat16))
    woT = _pack_pcf(np.asarray(Wo, np.float32).T.astype(ml_dtypes.bfloat16))
    return [
        {"xs": xs_all[b], "xT": xT_all[b], "wqT": wqT, "wkT": wkT,
         "wv": wv_, "woT": woT}
        for b in range(N_CORES)
    ]


def kernel(x, Wq, Wk, Wv, Wo):
    from concourse import bass_utils

    nc = _get_program()
    in_maps = prep_inputs(x, Wq, Wk, Wv, Wo)
    res = bass_utils.run_bass_kernel_spmd(nc, in_maps, core_ids=list(range(N_CORES)))
    outT_all = np.stack([res.results[b]["outT"] for b in range(N_CORES)], axis=0)
    return np.ascontiguousarray(np.transpose(outT_all, (0, 2, 1)))



# revision 15
# speedup vs baseline: 1.0056x; 1.0056x over previous
"""Trainium2 Bass kernel for nn_Attention_89670327206161 (Gram restructure).

The reference contracts attention scores over the *sequence* axis, so per
head the score matrix is only (dh x dh) = 64x64:
    scores_h = K_h^T Q_h / 8 = Wk_h (x^T x) Wq_h^T / 8
    out      = x . Wv^T . blockdiag(softmax(scores)) . Wo^T
The whole layer therefore collapses to GEMMs around one 1024x1024 Gram
matrix instead of three projections + attention + out-projection:
    G  = x^T x          (symmetric: pass A = cols 0:512 all rows,
                         pass B = bottom-right quadrant, top-right
                         quadrant mirrored with PE transposes)
    T2 = G Wq^T         (G's symmetry supplies the lhsT blocks; the 1/8
                         score scale is folded into the wq upload)
    scores_p = Wk_pair T2   (256-wide rhs keeps fp32r at full rate)
    P  = softmax_rows(scores)   (max-subtracted exp per 64x64 block;
                                 exp's accum_out gives the row sum free)
    U^T = BD(P)^T Wv-rows;  M = U Wo^T;  outT = M-blocks^T @ xT
The pre-softmax path (G, T2, scores) runs in fp32r -- softmax amplifies
logit error (|logits| reach ~140) so bf16 there is fatal. The
post-softmax path (P, Wv, U, Wo, M, xT) is plain linear algebra with
plenty of tolerance headroom, so it runs in bf16: 1024-wide moving
operands, half the DMA bytes, LDWEIGHTS hidden under longer matmuls.

Sharding: pure data parallelism -- one batch element per core, no
collectives. Host supplies x twice (seq-major fp32 for G, feature-major
bf16 for the final pass); output returns feature-major fp32, transposed
on host.

DMA choreography (the xa stream must never starve; TileContext schedules
by data deps only, so WAW gate copies into each DMA's destination pin
transfer start times):
  sync queue   : xa seq tiles, xb half tiles, xt blocks, out blocks.
  scalar queue : wq half0 behind xa tile 26; wq half1 + wk + wv + wo +
                 xt0 behind pass B's first copy-out -- all land inside
                 T2's DMA-quiet window, before their consumers.
  gpsimd queue : only gate copies and softmax scale muls, so the softmax
                 critical path is never queued behind DMA work.
"""

import numpy as np

HEADS = 16
B, S, D = 8, 4096, 1024
P = 128
NKC = D // P             # 8 chunks of 128 along D
NT = S // P              # 32 seq tiles
NPAIR = HEADS // 2       # 8 head pairs -> 128-wide blocks
N_CORES = 8

_PROGRAM = None


def _ts(i, n):
    return slice(i * n, (i + 1) * n)


def _build_program(reps=1):
    import concourse.bacc as bacc
    import concourse.mybir as mybir
    import concourse.tile as tile
    from concourse.masks import make_identity

    f32 = mybir.dt.float32
    f32r = mybir.dt.float32r
    bf16 = mybir.dt.bfloat16
    EXP = mybir.ActivationFunctionType.Exp
    X = mybir.AxisListType.X

    nc = bacc.Bacc(trn_type="TRN2", debug=False, num_devices=N_CORES)

    # weights and xT arrive pre-packed in SBUF layout ([partition, ...]
    # with long contiguous per-partition rows): the natural "(c p) o"
    # rearrangement produces 512-byte DMA descriptors, which run
    # descriptor-rate-bound at ~21 GB/s (measured: 2 MB of wq took
    # 11.5 us across 4160 descriptors)
    xs = nc.dram_tensor("xs", [S, D], f32r, kind="ExternalInput")
    xT = nc.dram_tensor("xT", [NKC, P, NKC * 512], bf16, kind="ExternalInput")
    wqT = nc.dram_tensor("wqT", [P, NKC * D], f32r, kind="ExternalInput")
    wkT = nc.dram_tensor("wkT", [P, NKC * D], f32r, kind="ExternalInput")
    wv = nc.dram_tensor("wv", [P, NKC * D], bf16, kind="ExternalInput")
    woT = nc.dram_tensor("woT", [P, NKC * D], bf16, kind="ExternalInput")
    outT = nc.dram_tensor("outT", [D, S], f32, kind="ExternalOutput")

    xs_ap = xs.ap()
    xTr = xT.ap().rearrange("b p (c s) -> b p c s", c=NKC)
    wqTr = wqT.ap().rearrange("p (c o) -> p c o", c=NKC)
    wkTr = wkT.ap().rearrange("p (c o) -> p c o", c=NKC)
    wvr = wv.ap().rearrange("p (r c) -> p r c", r=NKC)
    woTr = woT.ap().rearrange("p (c o) -> p c o", c=NKC)
    outTr = outT.ap().rearrange("(c p) s -> p c s", p=P)

    with tile.TileContext(nc) as tc:
      with (
          tc.tile_pool(name="const", bufs=1) as const_pool,
          tc.tile_pool(name="persist", bufs=1) as persist_pool,
          tc.tile_pool(name="smx", bufs=4) as smx_pool,
          tc.tile_pool(name="t2u", bufs=1) as t2u_pool,
          tc.tile_pool(name="arena", bufs=1) as arena,
          tc.tile_pool(name="wvwo", bufs=1) as wvwo_pool,
      ):
        zero_sb = const_pool.tile([P, 512], f32r, tag="zero")
        ident_raw = const_pool.tile([P, P], f32, tag="identr")
        ident = const_pool.tile([P, P], f32r, tag="ident")

        nc.vector.memset(zero_sb[:].bitcast(f32), 0.0)
        # affine_select output isn't fp32r-rounded for the BIR verifier;
        # route it through a copy, which is
        make_identity(nc, ident_raw[:])
        nc.vector.tensor_copy(ident[:], ident_raw[:])

        # reps>1 re-executes the whole body (timing builds: the difference
        # between reps=2 and reps=1 cancels dispatch overhead exactly)
        for _rep in range(reps):
          p_all = persist_pool.tile([P, NPAIR, P], bf16, tag="pall")
          nc.vector.memset(p_all[:], 0.0)
          g_sb = arena.tile([P, NKC, D], f32r, tag="a")
          t2_sb = t2u_pool.tile([P, NKC, D], f32r, tag="t2")
          wv_sb = wvwo_pool.tile([P, NKC, D], bf16, tag="wv")
          wo_sb = wvwo_pool.tile([P, NKC, D], bf16, tag="wo")
          ut_sb = wvwo_pool.tile([P, NKC, D], bf16, tag="ut")

          with (
              tc.tile_pool(name="wq", bufs=1) as wq_pool,
              tc.tile_pool(name="wk", bufs=1) as wk_pool,
          ):
            wq_sb = wq_pool.tile([P, NKC, D], f32r, tag="wq")
            wk_sb = wk_pool.tile([P, NKC, D], f32r, tag="wk")

            # Triangular G: only lower blocks (i >= j) are computed; the
            # upper triangle is mirrored with PE transposes.  fp32r MMs
            # below 256-wide run at 1/4 rate, so the two narrowest rows
            # are widened to 256 (their extra 128-col block lands on the
            # (0,1)/(4,5) upper blocks directly, skipping those mirrors).
            WA = [256, 256, 384, 512, 512, 512, 512, 512]
            OA = [0, 256, 512, 1024, 1536, 2048, 2560, 3072]  # 7 banks
            WB = [256, 256, 384, 512]
            OB = [0, 256, 512, 1024]  # 3 banks
            # top-left-quadrant mirrors go first: T2 h0 groups m=0..3
            # depend only on these (+ pass A), and are emitted right
            # after pass B's matmuls to cover the copy-out latency
            EARLY = [(c, m) for m in range(4, 8) for c in range(4)] + [
                (0, 2), (1, 2), (0, 3), (1, 3), (2, 3)]
            LATE = [(4, 6), (4, 7), (5, 6), (5, 7), (6, 7)]  # pass B srcs
            with (
                tc.tile_pool(name="xa", bufs=4) as xa_pool,
                tc.tile_pool(name="xbp", bufs=4) as xb_pool,
            ):
                # Pass A runs in TWO psum pools with one tile per bank:
                # separate tiles keep the copy-outs independent (a shared
                # tile makes tile insert false WAW deps that serialize
                # them S<->V), and the split releases chunks 4-7's banks
                # (right side) early so pass B's pools allocate over them
                # after only 4 copies instead of all 8.
                with tc.tile_pool(name="gA2", bufs=1, space="PSUM") as gA2_pool:
                  with tc.tile_pool(name="gA1", bufs=1, space="PSUM",
                                    side="right") as gA1_pool:
                    ga2 = [gA2_pool.tile([P, 512], f32, tag=f"gl{b}", name=f"gl{b}")
                           for b in range(3)]
                    ga1 = [gA1_pool.tile([P, 512], f32, tag=f"gr{b}", name=f"gr{b}")
                           for b in range(4)]
                    # chunk -> (psum tile, column offset)
                    A_LOC = [(ga2[0], 0), (ga2[0], 256), (ga2[1], 0),
                             (ga2[2], 0)] + [(ga1[j], 0) for j in range(4)]
                    # HAM warm-up + has_written clear: one dummy per bank
                    for t in ga2 + ga1:
                        nc.tensor.matmul(
                            t[:], zero_sb[:, 0:P], zero_sb[:],
                            start=True, stop=False, skip_group_check=True,
                        )
                    for st in range(NT):
                        xa = xa_pool.tile([P, D], f32r, tag="xa")
                        nc.sync.dma_start(xa[:], xs_ap[_ts(st, P), :])
                        for ci in range(NKC):
                            t, off = A_LOC[ci]
                            nc.tensor.matmul(
                                t[:, off:off + WA[ci]],
                                xa[:, _ts(ci, P)], xa[:, 0:WA[ci]],
                                start=False, stop=(st == NT - 1),
                                skip_group_check=True,
                            )
                        if st == 26:
                            # WAW gate (hoist protection): wq half0 waits
                            # for tile 26, then rides the sync queue right
                            # behind the xa stream
                            nc.gpsimd.tensor_copy(
                                wq_sb[0:1, 0, 0:512], xa[0:1, 0:512])
                    # scalar queue: keeps the sync queue free for the
                    # xb stream (pass A's xa stream already saturates
                    # it; 2MB of wq here would stall pass B's start)
                    nc.scalar.dma_start(wq_sb[:, :, 0:512], wqTr[:, :, 0:512])
                    for j in range(4):
                        eng = nc.scalar.copy if j % 2 == 0 else nc.vector.tensor_copy
                        eng(g_sb[:, 4 + j, 0:512], ga1[j][:])
                  # gA1 released: pass B's pools can allocate
                  nc.scalar.copy(
                      g_sb[:, 0:2, 0:256],
                      ga2[0][:].rearrange("p (c o) -> p c o", c=2))
                  nc.vector.tensor_copy(g_sb[:, 2, 0:384], ga2[1][:, 0:384])
                  nc.scalar.copy(g_sb[:, 3, 0:512], ga2[2][:])

                # ---- G pass B: lower-triangle cols 512:1024, rows 4-7,
                # with the pass-A-sourced mirrors riding the PE between
                # the DMA-paced pass-B tiles.  The T2 psum pool opens
                # here (3+3+2 = 8 banks) so the first T2 h0 groups can
                # fill the PE while pass B's copy-outs/gates drain.
                with tc.tile_pool(name="big", bufs=3, space="PSUM") as big_pool:

                  def t2_group(h, m):
                      ps = big_pool.tile([P, 512], f32, tag="bps")
                      for c in range(NKC):
                          nc.tensor.matmul(
                              ps[:], g_sb[:, c, _ts(m, P)],
                              wq_sb[:, c, _ts(h, 512)],
                              start=(c == 0), stop=(c == NKC - 1),
                          )
                      eng = nc.scalar.copy if m % 2 == 0 else nc.vector.tensor_copy
                      eng(t2_sb[:, m, _ts(h, 512)], ps[:])

                  with (
                    tc.tile_pool(name="gB", bufs=1, space="PSUM",
                                 side="right") as gB_pool,
                    tc.tile_pool(name="tr", bufs=2, space="PSUM",
                                 side="right") as tr_pool,
                  ):
                    def mirror(c, m, k):
                        # g_sb[:, c, m-block] = (g_sb[:, m, c-block])^T
                        t_ps = tr_pool.tile([P, P], f32r, tag="tr")
                        nc.tensor.transpose(
                            t_ps[:], g_sb[:, m, _ts(c, P)], ident[:])
                        eng = nc.scalar.copy if k % 2 == 0 else nc.vector.tensor_copy
                        eng(g_sb[:, c, _ts(m, P)], t_ps[:])

                    gb = [gB_pool.tile([P, 512], f32, tag=f"gb{b}", name=f"gb{b}")
                          for b in range(3)]
                    # chunk (4+j) -> (psum tile, column offset)
                    B_LOC = [(gb[0], 0), (gb[0], 256), (gb[1], 0), (gb[2], 0)]
                    # start=True clears has_written for the WHOLE bank, so
                    # banks shared by two row-chunks must be dummy-cleared
                    # once and then only accumulated into (start=False)
                    for t in gb:
                        nc.tensor.matmul(
                            t[:], zero_sb[:, 0:P], zero_sb[:],
                            start=True, stop=False, skip_group_check=True,
                        )
                    mi = 0
                    for st in range(NT):
                        xb = xb_pool.tile([P, 512], f32r, tag="xb")
                        nc.sync.dma_start(xb[:], xs_ap[_ts(st, P), 512:D])
                        for j in range(4):
                            t, off = B_LOC[j]
                            nc.tensor.matmul(
                                t[:, off:off + WB[j]],
                                xb[:, _ts(j, P)], xb[:, 0:WB[j]],
                                start=False, stop=(st == NT - 1),
                                skip_group_check=True,
                            )
                        if st >= 4 and mi < len(EARLY):
                            mirror(*EARLY[mi], mi)
                            mi += 1
                    # copy-outs, chunk 7 first: the weight-DMA gates key
                    # on it; per-bank tiles keep these independent
                    nc.scalar.copy(g_sb[:, 7, 512:1024], gb[2][:])
                    nc.vector.tensor_copy(g_sb[:, 6, 512:896], gb[1][:, 0:384])
                    nc.scalar.copy(
                        g_sb[:, 4:6, 512:768],
                        gb[0][:].rearrange("p (c o) -> p c o", c=2))

                    # WAW gates on chunk 7's copy-out: the remaining
                    # weights + xt0 flow during T2's DMA-quiet window
                    nc.gpsimd.tensor_copy(
                        wq_sb[0:1, 0, 512:D], g_sb[0:1, 7, 512:D])
                    nc.gpsimd.tensor_copy(
                        wk_sb[0:1, 0, :], g_sb[0:1, 7, :])
                    nc.gpsimd.tensor_copy(
                        wv_sb[0:1, 0, :], g_sb[0:1, 7, :])
                    nc.gpsimd.tensor_copy(
                        wo_sb[0:1, 0, :], g_sb[0:1, 7, :])
                    nc.sync.dma_start(wq_sb[:, :, 512:D], wqTr[:, :, 512:D])
                    nc.sync.dma_start(wk_sb[:, :, 0:512], wkTr[:, :, 0:512])
                    nc.sync.dma_start(wk_sb[:, :, 512:D], wkTr[:, :, 512:D])
                    nc.sync.dma_start(wv_sb[:], wvr[:])
                    nc.sync.dma_start(wo_sb[:], woTr[:])

                    # T2 h0 m=0..3 (pass A + quadrant mirrors only): PE
                    # work while the copy-outs, gates and weight DMAs run
                    for m in range(4):
                        t2_group(0, m)

                    for k, cm in enumerate(LATE):
                        mirror(*cm, k)

                  # ---- T2 / scores / softmax era
                  with tc.tile_pool(name="sc", bufs=4, space="PSUM") as sc_pool:
                    for m in range(4, NKC):
                        t2_group(0, m)

                    def softmax_half(ps, off, p, hf, rec):
                        # scores arrive pre-scaled by 1/8 but still reach
                        # |149|, so per-row max subtraction is mandatory.
                        # exp's accum_out yields the row sum for free.
                        # exp writes the UNNORMALIZED weights straight to
                        # p_all; the 1/rowsum lands on wv instead (one
                        # in-place scale per pair) so no per-block
                        # normalize op sits on the softmax critical path
                        # (the old gpsimd mul cost 1.16us per call and
                        # serialized the whole era).
                        rows = slice(64 * hf, 64 * hf + 64)
                        cols = slice(off + 64 * hf, off + 64 * hf + 64)
                        mx = smx_pool.tile([P, 1], f32, tag="mx")
                        nc.vector.reduce_max(mx[rows, 0:1], ps[rows, cols], axis=X, negate=True)
                        den = smx_pool.tile([P, 1], f32, tag="den")
                        nc.scalar.activation(
                            p_all[rows, p, 64 * hf:64 * hf + 64],
                            ps[rows, cols], EXP,
                            bias=mx[rows, 0:1], accum_out=den[rows, 0:1],
                        )
                        nc.vector.reciprocal(rec[rows, 0:1], den[rows, 0:1])

                    def ut_pair(p):
                        # U^T[pair-rows] = E_pair^T @ (diag(rec) Wv)[pair];
                        # emitted one pair behind softmax so the PE never
                        # waits on the softmax chain
                        for h in range(2):
                            ps = big_pool.tile([P, 512], f32, tag="bps")
                            nc.tensor.matmul(
                                ps[:], p_all[:, p, :], wv_sb[:, p, _ts(h, 512)],
                                start=True, stop=True,
                            )
                            eng = nc.scalar.copy if h == 0 else nc.vector.tensor_copy
                            eng(ut_sb[:, p, _ts(h, 512)], ps[:])

                    def scores_softmax(p):
                        c0 = min(p * P, D - 256)
                        ps = sc_pool.tile([P, 256], f32, tag="sc")
                        for m in range(NKC):
                            nc.tensor.matmul(
                                ps[:], wk_sb[:, m, _ts(p, P)],
                                t2_sb[:, m, c0:c0 + 256],
                                start=(m == 0), stop=(m == NKC - 1),
                            )
                        off = p * P - c0
                        rec = smx_pool.tile([P, 1], f32, tag="rec")
                        softmax_half(ps, off, p, 0, rec)
                        softmax_half(ps, off, p, 1, rec)
                        # fold the softmax row normalization into this
                        # pair's private wv rows (in place, off the PE)
                        nc.vector.tensor_scalar_mul(
                            wv_sb[:, p, :], wv_sb[:, p, :], rec[:, 0:1])

                    # T2 h1 with pairs 0-3 interleaved: their softmax chains
                    # finish under T2 compute instead of stalling U^T later
                    for m in range(NKC):
                        t2_group(1, m)
                        if m % 2 == 1:
                            scores_softmax(m // 2)
                    for p in range(4, NPAIR):
                        scores_softmax(p)
                        ut_pair(p - 4)
                    for p in range(4, NPAIR):
                        ut_pair(p)

          # ---- M / outT era (bf16 operands; PSUM outputs are
          # bank-limited to 512 fp32, so groups stay 512 wide)
          m_sb = arena.tile([P, NKC, D], bf16, tag="a")
          with (
              tc.tile_pool(name="xt", bufs=2) as xt_pool,
              tc.tile_pool(name="ot", bufs=8) as ot_pool,
              tc.tile_pool(name="bigB", bufs=4, space="PSUM") as bigB_pool,
          ):
              # xt0 gated (WAW) on T2h0's copy-out
              xt0 = xt_pool.tile([P, NKC, 512], bf16, tag="xt")
              nc.gpsimd.tensor_copy(xt0[0:1, 0, 0:1], t2_sb[0:1, 0, 0:1])
              nc.sync.dma_start(xt0[:], xTr[0])

              # ---- M = U @ Wo^T
              for h in range(2):
                  for a in range(NKC):
                      ps = bigB_pool.tile([P, 512], f32, tag="bps")
                      for c in range(NKC):
                          nc.tensor.matmul(
                              ps[:], ut_sb[:, c, _ts(a, P)],
                              wo_sb[:, c, _ts(h, 512)],
                              start=(c == 0), stop=(c == NKC - 1),
                          )
                      eng = nc.scalar.copy if a % 2 == 0 else nc.vector.tensor_copy
                      eng(m_sb[:, a, _ts(h, 512)], ps[:])

              # ---- outT = M-blocks^T @ xT, streamed in 512-seq blocks
              for sb in range(NKC):
                  if sb == 0:
                      xt = xt0
                  else:
                      xt = xt_pool.tile([P, NKC, 512], bf16, tag="xt")
                      nc.sync.dma_start(xt[:], xTr[sb])
                  for oc in range(NKC):
                      ps = bigB_pool.tile([P, 512], f32, tag="bps")
                      for ci in range(NKC):
                          nc.tensor.matmul(
                              ps[:], m_sb[:, ci, _ts(oc, P)], xt[:, ci, :],
                              start=(ci == 0), stop=(ci == NKC - 1),
                          )
                      ot = ot_pool.tile([P, 512], f32, tag="ot")
                      eng = nc.scalar.copy if oc % 2 == 0 else nc.vector.tensor_copy
                      eng(ot[:], ps[:])
                      # two queues: one lags ~2 transfers behind the MM
                      # stream and exposes the drain at the kernel tail
                      dq = nc.sync if oc % 2 == 0 else nc.scalar
                      dq.dma_start(outTr[:, oc, _ts(sb, 512)], ot[:])

    nc.compile()
    return nc


def _get_program():
    global _PROGRAM
    if _PROGRAM is None:
        _PROGRAM = _build_program()
    return _PROGRAM


def _pack_pcf(a):
    # [(c p), f] -> [p, (c f)]: SBUF layout, long contiguous DMA rows
    c = a.shape[0] // P
    return np.ascontiguousarray(
        a.reshape(c, P, a.shape[1]).transpose(1, 0, 2).reshape(P, -1))


def prep_inputs(x, Wq, Wk, Wv, Wo):
    import ml_dtypes

    x = np.asarray(x, np.float32)
    xs_all = np.ascontiguousarray(x)
    # xT packed per 512-seq block: [sb, p, (c s)]
    xT_all = np.ascontiguousarray(
        np.transpose(x, (0, 2, 1)).astype(ml_dtypes.bfloat16)
        .reshape(B, NKC, P, NKC, 512).transpose(0, 3, 2, 1, 4)
        .reshape(B, NKC, P, NKC * 512))
    # 1/8 score scale folded into wq so softmax needs no scale pass
    wqT = _pack_pcf(np.asarray(Wq, np.float32).T * np.float32(0.125))
    wkT = _pack_pcf(np.asarray(Wk, np.float32).T)
    wv_ = _pack_pcf(np.asarray(Wv, np.float32).astype(ml_dtypes.bfloat16))
    woT = _pack_pcf(np.asarray(Wo, np.float32).T.astype(ml_dtypes.bfloat16))
    return [
        {"xs": xs_all[b], "xT": xT_all[b], "wqT": wqT, "wkT": wkT,
         "wv": wv_, "woT": woT}
        for b in range(N_CORES)
    ]


def kernel(x, Wq, Wk, Wv, Wo):
    from concourse import bass_utils

    nc = _get_program()
    in_maps = prep_inputs(x, Wq, Wk, Wv, Wo)
    res = bass_utils.run_bass_kernel_spmd(nc, in_maps, core_ids=list(range(N_CORES)))
    outT_all = np.stack([res.results[b]["outT"] for b in range(N_CORES)], axis=0)
    return np.ascontiguousarray(np.transpose(outT_all, (0, 2, 1)))



# revision 16
# speedup vs baseline: 1.0194x; 1.0137x over previous
"""Trainium2 Bass kernel for nn_Attention_89670327206161 (Gram restructure).

The reference contracts attention scores over the *sequence* axis, so per
head the score matrix is only (dh x dh) = 64x64:
    scores_h = K_h^T Q_h / 8 = Wk_h (x^T x) Wq_h^T / 8
    out      = x . Wv^T . blockdiag(softmax(scores)) . Wo^T
The whole layer therefore collapses to GEMMs around one 1024x1024 Gram
matrix instead of three projections + attention + out-projection:
    G  = x^T x          (symmetric: pass A = cols 0:512 all rows,
                         pass B = bottom-right quadrant, top-right
                         quadrant mirrored with PE transposes)
    T2 = G Wq^T         (G's symmetry supplies the lhsT blocks; the 1/8
                         score scale is folded into the wq upload)
    scores_p = Wk_pair T2   (256-wide rhs keeps fp32r at full rate)
    P  = softmax_rows(scores)   (max-subtracted exp per 64x64 block;
                                 exp's accum_out gives the row sum free)
    U^T = BD(P)^T Wv-rows;  M = U Wo^T;  outT = M-blocks^T @ xT
The pre-softmax path (G, T2, scores) runs in fp32r -- softmax amplifies
logit error (|logits| reach ~140) so bf16 there is fatal. The
post-softmax path (P, Wv, U, Wo, M, xT) is plain linear algebra with
plenty of tolerance headroom, so it runs in bf16: 1024-wide moving
operands, half the DMA bytes, LDWEIGHTS hidden under longer matmuls.

Sharding: pure data parallelism -- one batch element per core, no
collectives. Host supplies x twice (seq-major fp32 for G, feature-major
bf16 for the final pass); output returns feature-major fp32, transposed
on host.

DMA choreography (the xa stream must never starve; TileContext schedules
by data deps only, so WAW gate copies into each DMA's destination pin
transfer start times):
  sync queue   : xa seq tiles, xb half tiles, xt blocks, out blocks.
  scalar queue : wq half0 behind xa tile 26; wq half1 + wk + wv + wo +
                 xt0 behind pass B's first copy-out -- all land inside
                 T2's DMA-quiet window, before their consumers.
  gpsimd queue : only gate copies and softmax scale muls, so the softmax
                 critical path is never queued behind DMA work.
"""

import numpy as np

HEADS = 16
B, S, D = 8, 4096, 1024
P = 128
NKC = D // P             # 8 chunks of 128 along D
NT = S // P              # 32 seq tiles
NPAIR = HEADS // 2       # 8 head pairs -> 128-wide blocks
N_CORES = 8

_PROGRAM = None


def _ts(i, n):
    return slice(i * n, (i + 1) * n)


def _build_program(reps=1):
    import concourse.bacc as bacc
    import concourse.mybir as mybir
    import concourse.tile as tile
    from concourse.masks import make_identity

    f32 = mybir.dt.float32
    f32r = mybir.dt.float32r
    bf16 = mybir.dt.bfloat16
    EXP = mybir.ActivationFunctionType.Exp
    X = mybir.AxisListType.X

    nc = bacc.Bacc(trn_type="TRN2", debug=False, num_devices=N_CORES)

    # weights and xT arrive pre-packed in SBUF layout ([partition, ...]
    # with long contiguous per-partition rows): the natural "(c p) o"
    # rearrangement produces 512-byte DMA descriptors, which run
    # descriptor-rate-bound at ~21 GB/s (measured: 2 MB of wq took
    # 11.5 us across 4160 descriptors)
    xs = nc.dram_tensor("xs", [S, D], f32r, kind="ExternalInput")
    xT = nc.dram_tensor("xT", [NKC, P, NKC * 512], bf16, kind="ExternalInput")
    wqT = nc.dram_tensor("wqT", [P, NKC * D], f32r, kind="ExternalInput")
    wkT = nc.dram_tensor("wkT", [P, NKC * D], f32r, kind="ExternalInput")
    wv = nc.dram_tensor("wv", [P, NKC * D], bf16, kind="ExternalInput")
    woT = nc.dram_tensor("woT", [P, NKC * D], bf16, kind="ExternalInput")
    outT = nc.dram_tensor("outT", [D, S], bf16, kind="ExternalOutput")

    xs_ap = xs.ap()
    xTr = xT.ap().rearrange("b p (c s) -> b p c s", c=NKC)
    wqTr = wqT.ap().rearrange("p (c o) -> p c o", c=NKC)
    wkTr = wkT.ap().rearrange("p (c o) -> p c o", c=NKC)
    wvr = wv.ap().rearrange("p (r c) -> p r c", r=NKC)
    woTr = woT.ap().rearrange("p (c o) -> p c o", c=NKC)
    outTr = outT.ap().rearrange("(c p) s -> p c s", p=P)

    with tile.TileContext(nc) as tc:
      with (
          tc.tile_pool(name="const", bufs=1) as const_pool,
          tc.tile_pool(name="persist", bufs=1) as persist_pool,
          tc.tile_pool(name="smx", bufs=4) as smx_pool,
          tc.tile_pool(name="t2u", bufs=1) as t2u_pool,
          tc.tile_pool(name="arena", bufs=1) as arena,
          tc.tile_pool(name="wvwo", bufs=1) as wvwo_pool,
      ):
        zero_sb = const_pool.tile([P, 512], f32r, tag="zero")
        ident_raw = const_pool.tile([P, P], f32, tag="identr")
        ident = const_pool.tile([P, P], f32r, tag="ident")

        nc.vector.memset(zero_sb[:].bitcast(f32), 0.0)
        # affine_select output isn't fp32r-rounded for the BIR verifier;
        # route it through a copy, which is
        make_identity(nc, ident_raw[:])
        nc.vector.tensor_copy(ident[:], ident_raw[:])

        # reps>1 re-executes the whole body (timing builds: the difference
        # between reps=2 and reps=1 cancels dispatch overhead exactly)
        for _rep in range(reps):
          p_all = persist_pool.tile([P, NPAIR, P], bf16, tag="pall")
          nc.vector.memset(p_all[:], 0.0)
          g_sb = arena.tile([P, NKC, D], f32r, tag="a")
          t2_sb = t2u_pool.tile([P, NKC, D], f32r, tag="t2")
          wv_sb = wvwo_pool.tile([P, NKC, D], bf16, tag="wv")
          wo_sb = wvwo_pool.tile([P, NKC, D], bf16, tag="wo")
          ut_sb = wvwo_pool.tile([P, NKC, D], bf16, tag="ut")

          with (
              tc.tile_pool(name="wq", bufs=1) as wq_pool,
              tc.tile_pool(name="wk", bufs=1) as wk_pool,
          ):
            wq_sb = wq_pool.tile([P, NKC, D], f32r, tag="wq")
            wk_sb = wk_pool.tile([P, NKC, D], f32r, tag="wk")

            # Triangular G: only lower blocks (i >= j) are computed; the
            # upper triangle is mirrored with PE transposes.  fp32r MMs
            # below 256-wide run at 1/4 rate, so the two narrowest rows
            # are widened to 256 (their extra 128-col block lands on the
            # (0,1)/(4,5) upper blocks directly, skipping those mirrors).
            WA = [256, 256, 384, 512, 512, 512, 512, 512]
            OA = [0, 256, 512, 1024, 1536, 2048, 2560, 3072]  # 7 banks
            WB = [256, 256, 384, 512]
            OB = [0, 256, 512, 1024]  # 3 banks
            # top-left-quadrant mirrors go first: T2 h0 groups m=0..3
            # depend only on these (+ pass A), and are emitted right
            # after pass B's matmuls to cover the copy-out latency
            EARLY = [(c, m) for m in range(4, 8) for c in range(4)] + [
                (0, 2), (1, 2), (0, 3), (1, 3), (2, 3)]
            LATE = [(4, 6), (4, 7), (5, 6), (5, 7), (6, 7)]  # pass B srcs
            with (
                tc.tile_pool(name="xa", bufs=4) as xa_pool,
                tc.tile_pool(name="xbp", bufs=4) as xb_pool,
            ):
                # Pass A runs in TWO psum pools with one tile per bank:
                # separate tiles keep the copy-outs independent (a shared
                # tile makes tile insert false WAW deps that serialize
                # them S<->V), and the split releases chunks 4-7's banks
                # (right side) early so pass B's pools allocate over them
                # after only 4 copies instead of all 8.
                with tc.tile_pool(name="gA2", bufs=1, space="PSUM") as gA2_pool:
                  with tc.tile_pool(name="gA1", bufs=1, space="PSUM",
                                    side="right") as gA1_pool:
                    ga2 = [gA2_pool.tile([P, 512], f32, tag=f"gl{b}", name=f"gl{b}")
                           for b in range(3)]
                    ga1 = [gA1_pool.tile([P, 512], f32, tag=f"gr{b}", name=f"gr{b}")
                           for b in range(4)]
                    # chunk -> (psum tile, column offset)
                    A_LOC = [(ga2[0], 0), (ga2[0], 256), (ga2[1], 0),
                             (ga2[2], 0)] + [(ga1[j], 0) for j in range(4)]
                    # HAM warm-up + has_written clear: one dummy per bank
                    for t in ga2 + ga1:
                        nc.tensor.matmul(
                            t[:], zero_sb[:, 0:P], zero_sb[:],
                            start=True, stop=False, skip_group_check=True,
                        )
                    xb_pre = []
                    for st in range(NT):
                        xa = xa_pool.tile([P, D], f32r, tag="xa")
                        nc.sync.dma_start(xa[:], xs_ap[_ts(st, P), :])
                        if st in (8, 10, 12, 14):
                            # prefetch pass B's first tiles into the sync
                            # ring's mid-pass slack (pass A is PE-bound)
                            k = len(xb_pre)
                            xbt = xb_pool.tile([P, 512], f32r, tag="xb")
                            nc.sync.dma_start(xbt[:], xs_ap[_ts(k, P), 512:D])
                            xb_pre.append(xbt)
                        for ci in range(NKC):
                            t, off = A_LOC[ci]
                            nc.tensor.matmul(
                                t[:, off:off + WA[ci]],
                                xa[:, _ts(ci, P)], xa[:, 0:WA[ci]],
                                start=False, stop=(st == NT - 1),
                                skip_group_check=True,
                            )
                        if st == 26:
                            # WAW gate (hoist protection): wq half0 waits
                            # for tile 26, then rides the sync queue right
                            # behind the xa stream
                            nc.gpsimd.tensor_copy(
                                wq_sb[0:1, 0, 0:512], xa[0:1, 0:512])
                    # scalar queue: keeps the sync queue free for the
                    # xb stream (pass A's xa stream already saturates
                    # it; 2MB of wq here would stall pass B's start)
                    nc.scalar.dma_start(wq_sb[:, :, 0:512], wqTr[:, :, 0:512])
                    for j in range(4):
                        eng = nc.scalar.copy if j % 2 == 0 else nc.vector.tensor_copy
                        eng(g_sb[:, 4 + j, 0:512], ga1[j][:])
                  # gA1 released: pass B's pools can allocate
                  nc.scalar.copy(
                      g_sb[:, 0:2, 0:256],
                      ga2[0][:].rearrange("p (c o) -> p c o", c=2))
                  nc.vector.tensor_copy(g_sb[:, 2, 0:384], ga2[1][:, 0:384])
                  nc.scalar.copy(g_sb[:, 3, 0:512], ga2[2][:])

                # ---- G pass B: lower-triangle cols 512:1024, rows 4-7,
                # with the pass-A-sourced mirrors riding the PE between
                # the DMA-paced pass-B tiles.  The T2 psum pool opens
                # here (3+3+2 = 8 banks) so the first T2 h0 groups can
                # fill the PE while pass B's copy-outs/gates drain.
                with tc.tile_pool(name="big", bufs=3, space="PSUM") as big_pool:

                  def t2_group(h, m):
                      ps = big_pool.tile([P, 512], f32, tag="bps")
                      for c in range(NKC):
                          nc.tensor.matmul(
                              ps[:], g_sb[:, c, _ts(m, P)],
                              wq_sb[:, c, _ts(h, 512)],
                              start=(c == 0), stop=(c == NKC - 1),
                          )
                      eng = nc.scalar.copy if m % 2 == 0 else nc.vector.tensor_copy
                      eng(t2_sb[:, m, _ts(h, 512)], ps[:])

                  with (
                    tc.tile_pool(name="gB", bufs=1, space="PSUM",
                                 side="right") as gB_pool,
                    tc.tile_pool(name="tr", bufs=2, space="PSUM",
                                 side="right") as tr_pool,
                  ):
                    def mirror(c, m, k):
                        # g_sb[:, c, m-block] = (g_sb[:, m, c-block])^T
                        t_ps = tr_pool.tile([P, P], f32r, tag="tr")
                        nc.tensor.transpose(
                            t_ps[:], g_sb[:, m, _ts(c, P)], ident[:])
                        eng = nc.scalar.copy if k % 2 == 0 else nc.vector.tensor_copy
                        eng(g_sb[:, c, _ts(m, P)], t_ps[:])

                    gb = [gB_pool.tile([P, 512], f32, tag=f"gb{b}", name=f"gb{b}")
                          for b in range(3)]
                    # chunk (4+j) -> (psum tile, column offset)
                    B_LOC = [(gb[0], 0), (gb[0], 256), (gb[1], 0), (gb[2], 0)]
                    # start=True clears has_written for the WHOLE bank, so
                    # banks shared by two row-chunks must be dummy-cleared
                    # once and then only accumulated into (start=False)
                    for t in gb:
                        nc.tensor.matmul(
                            t[:], zero_sb[:, 0:P], zero_sb[:],
                            start=True, stop=False, skip_group_check=True,
                        )
                    mi = 0
                    for st in range(NT):
                        if st < len(xb_pre):
                            xb = xb_pre[st]
                        else:
                            xb = xb_pool.tile([P, 512], f32r, tag="xb")
                            nc.sync.dma_start(xb[:], xs_ap[_ts(st, P), 512:D])
                        for j in range(4):
                            t, off = B_LOC[j]
                            nc.tensor.matmul(
                                t[:, off:off + WB[j]],
                                xb[:, _ts(j, P)], xb[:, 0:WB[j]],
                                start=False, stop=(st == NT - 1),
                                skip_group_check=True,
                            )
                        if st >= 4 and mi < len(EARLY):
                            mirror(*EARLY[mi], mi)
                            mi += 1
                    # copy-outs, chunk 7 first: the weight-DMA gates key
                    # on it; per-bank tiles keep these independent
                    nc.scalar.copy(g_sb[:, 7, 512:1024], gb[2][:])
                    nc.vector.tensor_copy(g_sb[:, 6, 512:896], gb[1][:, 0:384])
                    nc.scalar.copy(
                        g_sb[:, 4:6, 512:768],
                        gb[0][:].rearrange("p (c o) -> p c o", c=2))

                    # WAW gates on chunk 7's copy-out: the remaining
                    # weights + xt0 flow during T2's DMA-quiet window
                    nc.gpsimd.tensor_copy(
                        wq_sb[0:1, 0, 512:D], g_sb[0:1, 7, 512:D])
                    nc.gpsimd.tensor_copy(
                        wk_sb[0:1, 0, :], g_sb[0:1, 7, :])
                    nc.gpsimd.tensor_copy(
                        wv_sb[0:1, 0, :], g_sb[0:1, 7, :])
                    nc.gpsimd.tensor_copy(
                        wo_sb[0:1, 0, :], g_sb[0:1, 7, :])
                    nc.sync.dma_start(wq_sb[:, :, 512:D], wqTr[:, :, 512:D])
                    nc.sync.dma_start(wk_sb[:, :, 0:512], wkTr[:, :, 0:512])
                    nc.sync.dma_start(wk_sb[:, :, 512:D], wkTr[:, :, 512:D])
                    nc.sync.dma_start(wv_sb[:], wvr[:])
                    nc.sync.dma_start(wo_sb[:], woTr[:])

                    # T2 h0 m=0..3 (pass A + quadrant mirrors only): PE
                    # work while the copy-outs, gates and weight DMAs run
                    for m in range(4):
                        t2_group(0, m)

                    for k, cm in enumerate(LATE):
                        mirror(*cm, k)

                  # ---- T2 / scores / softmax era
                  with tc.tile_pool(name="sc", bufs=4, space="PSUM") as sc_pool:
                    for m in range(4, NKC):
                        t2_group(0, m)

                    def softmax_half(ps, off, p, hf, rec):
                        # scores arrive pre-scaled by 1/8 but still reach
                        # |149|, so per-row max subtraction is mandatory.
                        # exp's accum_out yields the row sum for free.
                        # exp writes the UNNORMALIZED weights straight to
                        # p_all; the 1/rowsum lands on wv instead (one
                        # in-place scale per pair) so no per-block
                        # normalize op sits on the softmax critical path
                        # (the old gpsimd mul cost 1.16us per call and
                        # serialized the whole era).
                        rows = slice(64 * hf, 64 * hf + 64)
                        cols = slice(off + 64 * hf, off + 64 * hf + 64)
                        mx = smx_pool.tile([P, 1], f32, tag="mx")
                        nc.vector.reduce_max(mx[rows, 0:1], ps[rows, cols], axis=X, negate=True)
                        den = smx_pool.tile([P, 1], f32, tag="den")
                        nc.scalar.activation(
                            p_all[rows, p, 64 * hf:64 * hf + 64],
                            ps[rows, cols], EXP,
                            bias=mx[rows, 0:1], accum_out=den[rows, 0:1],
                        )
                        nc.vector.reciprocal(rec[rows, 0:1], den[rows, 0:1])

                    def ut_pair(p):
                        # U^T[pair-rows] = E_pair^T @ (diag(rec) Wv)[pair];
                        # emitted one pair behind softmax so the PE never
                        # waits on the softmax chain
                        for h in range(2):
                            ps = big_pool.tile([P, 512], f32, tag="bps")
                            nc.tensor.matmul(
                                ps[:], p_all[:, p, :], wv_sb[:, p, _ts(h, 512)],
                                start=True, stop=True,
                            )
                            eng = nc.scalar.copy if h == 0 else nc.vector.tensor_copy
                            eng(ut_sb[:, p, _ts(h, 512)], ps[:])

                    def scores_softmax(p):
                        c0 = min(p * P, D - 256)
                        ps = sc_pool.tile([P, 256], f32, tag="sc")
                        for m in range(NKC):
                            nc.tensor.matmul(
                                ps[:], wk_sb[:, m, _ts(p, P)],
                                t2_sb[:, m, c0:c0 + 256],
                                start=(m == 0), stop=(m == NKC - 1),
                            )
                        off = p * P - c0
                        rec = smx_pool.tile([P, 1], f32, tag="rec")
                        softmax_half(ps, off, p, 0, rec)
                        softmax_half(ps, off, p, 1, rec)
                        # fold the softmax row normalization into this
                        # pair's private wv rows (in place, off the PE)
                        nc.vector.tensor_scalar_mul(
                            wv_sb[:, p, :], wv_sb[:, p, :], rec[:, 0:1])

                    # T2 h1 with pairs 0-3 interleaved: their softmax chains
                    # finish under T2 compute instead of stalling U^T later
                    for m in range(NKC):
                        t2_group(1, m)
                        if m % 2 == 1:
                            scores_softmax(m // 2)
                    for p in range(4, NPAIR):
                        scores_softmax(p)
                        ut_pair(p - 4)
                    for p in range(4, NPAIR):
                        ut_pair(p)

          # ---- M / outT era (bf16 operands; PSUM outputs are
          # bank-limited to 512 fp32, so groups stay 512 wide)
          m_sb = arena.tile([P, NKC, D], bf16, tag="a")
          with (
              tc.tile_pool(name="xt", bufs=2) as xt_pool,
              tc.tile_pool(name="ot", bufs=8) as ot_pool,
              tc.tile_pool(name="bigB", bufs=4, space="PSUM") as bigB_pool,
          ):
              # xt0 gated (WAW) on T2h0's copy-out
              xt0 = xt_pool.tile([P, NKC, 512], bf16, tag="xt")
              nc.gpsimd.tensor_copy(xt0[0:1, 0, 0:1], t2_sb[0:1, 0, 0:1])
              nc.sync.dma_start(xt0[:], xTr[0])

              # ---- M = U @ Wo^T
              for h in range(2):
                  for a in range(NKC):
                      ps = bigB_pool.tile([P, 512], f32, tag="bps")
                      for c in range(NKC):
                          nc.tensor.matmul(
                              ps[:], ut_sb[:, c, _ts(a, P)],
                              wo_sb[:, c, _ts(h, 512)],
                              start=(c == 0), stop=(c == NKC - 1),
                          )
                      eng = nc.scalar.copy if a % 2 == 0 else nc.vector.tensor_copy
                      eng(m_sb[:, a, _ts(h, 512)], ps[:])

              # ---- outT = M-blocks^T @ xT, streamed in 512-seq blocks
              for sb in range(NKC):
                  if sb == 0:
                      xt = xt0
                  else:
                      xt = xt_pool.tile([P, NKC, 512], bf16, tag="xt")
                      nc.sync.dma_start(xt[:], xTr[sb])
                  for oc in range(NKC):
                      ps = bigB_pool.tile([P, 512], f32, tag="bps")
                      for ci in range(NKC):
                          nc.tensor.matmul(
                              ps[:], m_sb[:, ci, _ts(oc, P)], xt[:, ci, :],
                              start=(ci == 0), stop=(ci == NKC - 1),
                          )
                      ot = ot_pool.tile([P, 512], bf16, tag="ot")
                      eng = nc.scalar.copy if oc % 2 == 0 else nc.vector.tensor_copy
                      eng(ot[:], ps[:])
                      # two queues: one lags ~2 transfers behind the MM
                      # stream and exposes the drain at the kernel tail
                      dq = nc.sync if oc % 2 == 0 else nc.scalar
                      dq.dma_start(outTr[:, oc, _ts(sb, 512)], ot[:])

    nc.compile()
    return nc


def _get_program():
    global _PROGRAM
    if _PROGRAM is None:
        _PROGRAM = _build_program()
    return _PROGRAM


def _pack_pcf(a):
    # [(c p), f] -> [p, (c f)]: SBUF layout, long contiguous DMA rows
    c = a.shape[0] // P
    return np.ascontiguousarray(
        a.reshape(c, P, a.shape[1]).transpose(1, 0, 2).reshape(P, -1))


def prep_inputs(x, Wq, Wk, Wv, Wo):
    import ml_dtypes

    x = np.asarray(x, np.float32)
    xs_all = np.ascontiguousarray(x)
    # xT packed per 512-seq block: [sb, p, (c s)]
    xT_all = np.ascontiguousarray(
        np.transpose(x, (0, 2, 1)).astype(ml_dtypes.bfloat16)
        .reshape(B, NKC, P, NKC, 512).transpose(0, 3, 2, 1, 4)
        .reshape(B, NKC, P, NKC * 512))
    # 1/8 score scale folded into wq so softmax needs no scale pass
    wqT = _pack_pcf(np.asarray(Wq, np.float32).T * np.float32(0.125))
    wkT = _pack_pcf(np.asarray(Wk, np.float32).T)
    wv_ = _pack_pcf(np.asarray(Wv, np.float32).astype(ml_dtypes.bfloat16))
    woT = _pack_pcf(np.asarray(Wo, np.float32).T.astype(ml_dtypes.bfloat16))
    return [
        {"xs": xs_all[b], "xT": xT_all[b], "wqT": wqT, "wkT": wkT,
         "wv": wv_, "woT": woT}
        for b in range(N_CORES)
    ]


def kernel(x, Wq, Wk, Wv, Wo):
    from concourse import bass_utils

    nc = _get_program()
    in_maps = prep_inputs(x, Wq, Wk, Wv, Wo)
    res = bass_utils.run_bass_kernel_spmd(nc, in_maps, core_ids=list(range(N_CORES)))
    outT_all = np.stack([np.asarray(res.results[b]["outT"], np.float32)
                         for b in range(N_CORES)], axis=0)
    return np.ascontiguousarray(np.transpose(outT_all, (0, 2, 1)))

